# revision 1
# baseline (speedup 1.0000x reference)
"""Trainium2 Bass kernel: per-sample dynamic conv (KernelAggregation).

Problem: out[b] = conv2d(x[b], sum_n att[b,n]*W[n], pad=1) + (att @ bias)[b]
  x: (16, 256, 56, 56) f32, att: (16, 8), W: (8, 256, 256, 3, 3), bias: (8, 256)

Sharding: data-parallel over batch, 2 samples per core across 8 cores.

Per-core device kernel:
  1. Stream the (host pre-transposed) weight bank once from DRAM; mix both
     samples' dynamic conv weights on VectorE via scalar_tensor_tensor FMA
     (w_mix[s] += att[s,n] * W[n]), in matmul-ready [ci, (ky,kx,co)] layout.
  2. Conv as 9 shifted matmuls over a 58-stride zero-padded input image:
     out[co, p] += w_mix[ci, kp, co].T @ xpad[ci, p + dy*58+dx], accumulated
     in PSUM over 2 ci-chunks x 9 taps; N-tiles of 464 px (8 rows).
  3. ScalarE adds the mixed bias (Identity activation, per-partition bias)
     while copying PSUM -> SBUF; DMA result rows (dropping the 2 pad cols).

Matmul dtype is float32r (TF32-like, full PE rate at N>=256) by default;
set _MM_DTYPE = "float32" for exact-fp32 (4x slower PE).
"""

import numpy as np
from contextlib import ExitStack

B, DIM, H, W = 16, 256, 56, 56
NK, KS = 8, 3
NCORES = 8
SPC = B // NCORES          # samples per core
S = W + 2                  # padded row stride (58)
NPAD = S * S               # 3364
XP_LEN = NPAD + 4          # slack so shifted reads stay in-bounds
ROWS_PER_T = 8
NT = H // ROWS_PER_T       # 7 spatial tiles
NTILE = ROWS_PER_T * S     # 464 (= matmul moving dim, <=512 fp32)
CI_CH = DIM // 128         # 2
CO_CH = DIM // 128         # 2
KK = KS * KS               # 9

_MM_DTYPE = "float32r"     # "float32r" | "float32" | "bfloat16"


def _imports():
    try:
        import concourse.bass as bass  # noqa: F401
    except ImportError:
        import sys
        for p in ("/opt/trn_rl_repo",):
            if p not in sys.path:
                sys.path.insert(0, p)
    import concourse.bass as bass
    import concourse.tile as tile
    from concourse import mybir
    from concourse.bass_utils import run_bass_kernel_spmd
    return bass, tile, mybir, run_bass_kernel_spmd


NBANK = 3   # bank streaming buffers
NTMP = 4    # ACT->DVE scaled-weight staging buffers
NPS = 4     # PSUM tiles
NOUT = 4    # output staging buffers


def build_bass_raw(mm_dtype_name=None):
    bass, tile, mybir, _ = _imports()
    dt = mybir.dt
    mm_dtype = getattr(dt, mm_dtype_name or _MM_DTYPE)
    nc = bass.Bass()

    x = nc.dram_tensor("x", [SPC, DIM, H, W], mm_dtype, kind="ExternalInput")
    wbank = nc.dram_tensor("wbank", [NK, CI_CH, 128, KK * DIM], dt.float32,
                           kind="ExternalInput")
    attb = nc.dram_tensor("attb", [128, SPC * NK], dt.float32,
                          kind="ExternalInput")
    bmixT = nc.dram_tensor("bmixT", [128, CO_CH * SPC], dt.float32,
                           kind="ExternalInput")
    y = nc.dram_tensor("y", [SPC, DIM, H, W], dt.float32, kind="ExternalOutput")

    ctx = ExitStack()
    with ctx:
        sb = lambda shape, name: ctx.enter_context(
            nc.sbuf_tensor(name, shape, dt.float32))
        sbm = lambda shape, name: ctx.enter_context(
            nc.sbuf_tensor(name, shape, mm_dtype))
        att_sb = sb([128, SPC * NK], "att_sb")
        bmix_sb = sb([128, CO_CH * SPC], "bmix_sb")
        xp = [[sbm([128, XP_LEN], f"xp{s}_{c}") for c in range(CI_CH)]
              for s in range(SPC)]
        wmix = [[sbm([128, KK * DIM], f"wm{s}_{c}") for c in range(CI_CH)]
                for s in range(SPC)]
        bank = [sb([128, KK * DIM], f"bank{i}") for i in range(NBANK)]
        tmp = [sb([128, KK * DIM], f"tmp{i}") for i in range(NTMP)]
        ot = [sb([128, NTILE], f"ot{i}") for i in range(NOUT)]
        psum = [ctx.enter_context(nc.psum_tensor(f"ps{i}", [128, NTILE],
                                                 dt.float32))
                for i in range(NPS)]

        sem = lambda name: ctx.enter_context(nc.semaphore(name))
        sem_small = sem("sem_small")   # att/bmix DMA done (2x16)
        sem_ms = sem("sem_ms")         # DVE memsets done (1 each, 4)
        sem_x = sem("sem_x")           # x interior DMA done (4x16)
        sem_bank = sem("sem_bank")     # bank DMA k done at 16*(k+1)
        sem_scale = sem("sem_scale")   # ACT weight scale-muls (1 each, 32)
        sem_mixop = sem("sem_mixop")   # DVE wmix copy/adds (1 each, 32)
        sem_mm = sem("sem_mm")         # PE per-out-tile group done (1 ea, 28)
        sem_act = sem("sem_act")       # ACT out bias-copies (1 each, 28)
        sem_outdma = sem("sem_outdma")  # out DMA done (16 each, 28)

        Copy = mybir.ActivationFunctionType.Copy
        Ident = mybir.ActivationFunctionType.Identity

        # ---------------- DVE: memsets, then wmix accumulate
        for i, (s, c) in enumerate([(s, c) for s in range(SPC)
                                    for c in range(CI_CH)]):
            ms_ap = xp[s][c][:]
            if mm_dtype != dt.float32:
                ms_ap = ms_ap.bitcast(dt.float32)  # memset lacks f32r ISA
            nc.vector.memset(ms_ap, 0.0).then_inc(sem_ms, 1)
        j = 0
        for k in range(NK * CI_CH):
            n, c = divmod(k, CI_CH)
            for s in range(SPC):
                nc.vector.wait_ge(sem_scale, j + 1)
                t = tmp[j % NTMP][:]
                if n == 0:
                    nc.vector.tensor_copy(wmix[s][c][:], t).then_inc(
                        sem_mixop, 1)
                else:
                    nc.vector.tensor_add(wmix[s][c][:], wmix[s][c][:],
                                         t).then_inc(sem_mixop, 1)
                j += 1

        # ---------------- GPSIMD: all input DMAs
        nc.gpsimd.dma_start(att_sb[:], attb[:, :]).then_inc(sem_small, 16)
        nc.gpsimd.dma_start(bmix_sb[:], bmixT[:, :]).then_inc(sem_small, 16)
        for k in range(min(NBANK, NK * CI_CH)):
            n, c = divmod(k, CI_CH)
            nc.gpsimd.dma_start(bank[k % NBANK][:],
                                wbank[n, c, :, :]).then_inc(sem_bank, 16)
        for i, (s, c) in enumerate([(s, c) for s in range(SPC)
                                    for c in range(CI_CH)]):
            nc.gpsimd.wait_ge(sem_ms, i + 1)
            interior = xp[s][c][:, :NPAD].rearrange(
                "p (r u) -> p r u", u=S)[:, 1:1 + H, 1:1 + W]
            nc.gpsimd.dma_start(
                interior, x[s, c * 128:(c + 1) * 128, :, :]).then_inc(sem_x, 16)
        for k in range(NBANK, NK * CI_CH):
            n, c = divmod(k, CI_CH)
            nc.gpsimd.wait_ge(sem_scale, 2 * (k - NBANK) + 2)
            nc.gpsimd.dma_start(bank[k % NBANK][:],
                                wbank[n, c, :, :]).then_inc(sem_bank, 16)

        # ---------------- ACT: weight scale-muls, then out bias-copies
        nc.scalar.wait_ge(sem_small, 32)
        j = 0
        for k in range(NK * CI_CH):
            n, c = divmod(k, CI_CH)
            nc.scalar.wait_ge(sem_bank, 16 * (k + 1))
            for s in range(SPC):
                if j >= NTMP:
                    nc.scalar.wait_ge(sem_mixop, j - NTMP + 1)
                nc.scalar.activation(
                    tmp[j % NTMP][:], bank[k % NBANK][:],
                    Copy, scale=att_sb[:, s * NK + n: s * NK + n + 1],
                ).then_inc(sem_scale, 1)
                j += 1
        tiles = [(s, t, co) for s in range(SPC) for t in range(NT)
                 for co in range(CO_CH)]
        for ti, (s, t, co) in enumerate(tiles):
            nc.scalar.wait_ge(sem_mm, ti + 1)
            if ti >= NOUT:
                nc.scalar.wait_ge(sem_outdma, 16 * (ti - NOUT + 1))
            nc.scalar.activation(
                ot[ti % NOUT][:], psum[ti % NPS][:], Ident,
                bias=bmix_sb[:, co * SPC + s: co * SPC + s + 1],
            ).then_inc(sem_act, 1)

        # ---------------- PE: conv matmuls
        nc.tensor.wait_ge(sem_x, 16 * SPC * CI_CH)
        nc.tensor.wait_ge(sem_mixop, SPC * NK * CI_CH)
        for ti, (s, t, co) in enumerate(tiles):
            if ti >= NPS:
                nc.tensor.wait_ge(sem_act, ti - NPS + 1)
            for c in range(CI_CH):
                for kp in range(KK):
                    off = (kp // 3) * S + (kp % 3) + t * NTILE
                    lhsT = wmix[s][c][:, kp * DIM + co * 128:
                                      kp * DIM + co * 128 + 128]
                    rhs = xp[s][c][:, off: off + NTILE]
                    mm = nc.tensor.matmul(
                        psum[ti % NPS][:], lhsT, rhs,
                        start=(c == 0 and kp == 0),
                        stop=(c == CI_CH - 1 and kp == KK - 1))
            mm.then_inc(sem_mm, 1)

        # ---------------- SYNC: output DMAs
        for ti, (s, t, co) in enumerate(tiles):
            nc.sync.wait_ge(sem_act, ti + 1)
            src = ot[ti % NOUT][:].rearrange("p (r u) -> p r u", u=S)[:, :, 0:W]
            nc.sync.dma_start(
                y[s, co * 128:(co + 1) * 128,
                  t * ROWS_PER_T:(t + 1) * ROWS_PER_T, :], src,
            ).then_inc(sem_outdma, 16)
        nc.sync.wait_ge(sem_outdma, 16 * len(tiles))
    return nc




def prep_inputs(x, attention, weight, bias):
    """Host-side sharding + layout prep. Returns per-core input maps."""
    x = np.ascontiguousarray(np.asarray(x, dtype=np.float32))
    attention = np.asarray(attention, dtype=np.float32)
    weight = np.asarray(weight, dtype=np.float32)
    bias = np.asarray(bias, dtype=np.float32)

    # (n, co, ci, ky, kx) -> (n, ci, ky, kx, co) -> [n, ci_ch, 128, kk*co]
    wb = np.ascontiguousarray(weight.transpose(0, 2, 3, 4, 1)).reshape(
        NK, CI_CH, 128, KK * DIM)
    # att broadcast across partitions: [128, B*NK]
    attb_all = np.ascontiguousarray(
        np.repeat(attention.reshape(1, B * NK), 128, axis=0))
    # host-mixed bias: bm = att @ bias; bmixT[p, co*SPC+s] = bm[s0+s, co*128+p]
    bm = attention @ bias

    in_maps = []
    for cidx in range(NCORES):
        s0 = cidx * SPC
        in_maps.append({
            "x": np.ascontiguousarray(x[s0:s0 + SPC]),
            "wbank": wb,
            "attb": np.ascontiguousarray(
                attb_all[:, s0 * NK:(s0 + SPC) * NK]),
            "bmixT": np.ascontiguousarray(
                bm[s0:s0 + SPC].reshape(SPC, CO_CH, 128).transpose(
                    2, 1, 0)).reshape(128, CO_CH * SPC),
        })
    return in_maps




def run(x, attention, weight, bias, trace=False, mm_dtype_name=None, **kw):
    _, _, _, run_bass_kernel_spmd = _imports()
    nc = build_bass_raw(mm_dtype_name)
    in_maps = prep_inputs(x, attention, weight, bias)
    res = run_bass_kernel_spmd(nc, in_maps, list(range(NCORES)),
                               trace=trace, **kw)
    y = np.concatenate([res.results[i]["y"] for i in range(NCORES)], axis=0)
    return y.astype(np.float32), res


def kernel(x, attention, weight, bias):
    y, _ = run(x, attention, weight, bias)
    return y



# revision 2
# speedup vs baseline: 1.2458x; 1.2458x over previous
"""Trainium2 Bass kernel: per-sample dynamic conv (KernelAggregation).

Problem: out[b] = conv2d(x[b], sum_n att[b,n]*W[n], pad=1) + (att @ bias)[b]
  x: (16, 256, 56, 56) f32, att: (16, 8), W: (8, 256, 256, 3, 3), bias: (8, 256)

Sharding: data-parallel over batch, 2 samples per core across 8 cores.

The axon tunnel moves ~40 MB/s each way, so wall time is dominated by wire
bytes, not device compute (~0.2 ms/core). Design:
  * per-sample conv weights are mixed on the host (att @ bank, one sgemm)
    and shipped per-sample in bf16 (18.9 MB) instead of replicating the
    fp32 bank to all 8 cores (151 MB),
  * x ships as bf16 (25.7 MB, not 51.4), y returns as bf16,
  * the donated-zero output upload of run_bass_kernel_spmd's axon path is
    replaced by a persistent on-device dummy operand (the kernel writes
    every y element, so zero-init is unnecessary),
  * the shard_map jit is built once at module scope and reused, so warm
    calls pay only transfers + one dispatch.

Device kernel per core (2 samples): DMA x into a 58-stride zero-padded
SBUF image; conv = 9 shifted bf16 matmuls accumulated in PSUM over 2
ci-chunks; ScalarE adds the host-mixed bias while copying PSUM -> bf16
tiles; DMA out, dropping the pad columns.
"""

import numpy as np
from contextlib import ExitStack

B, DIM, H, W = 16, 256, 56, 56
NK, KS = 8, 3
NCORES = 8
SPC = B // NCORES          # samples per core
S = W + 2                  # padded row stride (58)
NPAD = S * S               # 3364
XP_LEN = NPAD + 4          # slack so shifted reads stay in-bounds (even)
ROWS_PER_T = 8
NT = H // ROWS_PER_T       # 7 spatial tiles
NTILE = ROWS_PER_T * S     # 464 (matmul moving dim)
CI_CH = DIM // 128         # 2
CO_CH = DIM // 128         # 2
KK = KS * KS               # 9

NPS = 4     # PSUM tiles
NOUT = 4    # output staging buffers


def _imports():
    try:
        import concourse.bass as bass  # noqa: F401
    except ImportError:
        import sys
        for p in ("/opt/trn_rl_repo",):
            if p not in sys.path:
                sys.path.insert(0, p)
    import concourse.bass as bass
    import concourse.tile as tile
    from concourse import mybir
    return bass, tile, mybir


def build_bass_raw():
    bass, tile, mybir = _imports()
    dt = mybir.dt
    nc = bass.Bass()

    x = nc.dram_tensor("x", [SPC, DIM, H, W], dt.bfloat16, kind="ExternalInput")
    wmixT = nc.dram_tensor("wmixT", [SPC, CI_CH, 128, KK * DIM], dt.bfloat16,
                           kind="ExternalInput")
    bmixT = nc.dram_tensor("bmixT", [128, CO_CH * SPC], dt.float32,
                           kind="ExternalInput")
    y = nc.dram_tensor("y", [SPC, DIM, H, W], dt.bfloat16,
                       kind="ExternalOutput")

    ctx = ExitStack()
    with ctx:
        sbm = lambda shape, name: ctx.enter_context(
            nc.sbuf_tensor(name, shape, dt.bfloat16))
        bmix_sb = ctx.enter_context(
            nc.sbuf_tensor("bmix_sb", [128, CO_CH * SPC], dt.float32))
        xp = [[sbm([128, XP_LEN], f"xp{s}_{c}") for c in range(CI_CH)]
              for s in range(SPC)]
        wm = [[sbm([128, KK * DIM], f"wm{s}_{c}") for c in range(CI_CH)]
              for s in range(SPC)]
        ot = [sbm([128, NTILE], f"ot{i}") for i in range(NOUT)]
        psum = [ctx.enter_context(nc.psum_tensor(f"ps{i}", [128, NTILE],
                                                 dt.float32))
                for i in range(NPS)]

        sem = lambda name: ctx.enter_context(nc.semaphore(name))
        sem_small = sem("sem_small")   # bmix DMA done (16)
        sem_ms = sem("sem_ms")         # DVE memsets done (1 each, 4)
        sem_x = sem("sem_x")           # x interior DMA done (4x16)
        sem_w = sem("sem_w")           # wmix DMA done (4x16)
        sem_mm = sem("sem_mm")         # PE per-out-tile group done (1 ea, 28)
        sem_act = sem("sem_act")       # ACT out bias-copies (1 each, 28)
        sem_outdma = sem("sem_outdma")  # out DMA done (16 each, 28)

        Ident = mybir.ActivationFunctionType.Identity

        # ---------------- DVE: zero the padded x images (borders matter)
        for i, (s, c) in enumerate([(s, c) for s in range(SPC)
                                    for c in range(CI_CH)]):
            nc.vector.memset(xp[s][c][:].bitcast(mybir.dt.float32),
                             0.0).then_inc(sem_ms, 1)

        # ---------------- GPSIMD: all input DMAs
        nc.gpsimd.dma_start(bmix_sb[:], bmixT[:, :]).then_inc(sem_small, 16)
        for s in range(SPC):
            for c in range(CI_CH):
                nc.gpsimd.dma_start(wm[s][c][:],
                                    wmixT[s, c, :, :]).then_inc(sem_w, 16)
        for i, (s, c) in enumerate([(s, c) for s in range(SPC)
                                    for c in range(CI_CH)]):
            nc.gpsimd.wait_ge(sem_ms, i + 1)
            interior = xp[s][c][:, :NPAD].rearrange(
                "p (r u) -> p r u", u=S)[:, 1:1 + H, 1:1 + W]
            nc.gpsimd.dma_start(
                interior, x[s, c * 128:(c + 1) * 128, :, :]).then_inc(sem_x, 16)

        # ---------------- PE: conv matmuls
        tiles = [(s, t, co) for s in range(SPC) for t in range(NT)
                 for co in range(CO_CH)]
        nc.tensor.wait_ge(sem_x, 16 * SPC * CI_CH)
        nc.tensor.wait_ge(sem_w, 16 * SPC * CI_CH)
        for ti, (s, t, co) in enumerate(tiles):
            if ti >= NPS:
                nc.tensor.wait_ge(sem_act, ti - NPS + 1)
            for c in range(CI_CH):
                for kp in range(KK):
                    off = (kp // 3) * S + (kp % 3) + t * NTILE
                    lhsT = wm[s][c][:, kp * DIM + co * 128:
                                    kp * DIM + co * 128 + 128]
                    rhs = xp[s][c][:, off: off + NTILE]
                    mm = nc.tensor.matmul(
                        psum[ti % NPS][:], lhsT, rhs,
                        start=(c == 0 and kp == 0),
                        stop=(c == CI_CH - 1 and kp == KK - 1))
            mm.then_inc(sem_mm, 1)

        # ---------------- ACT: bias-add + cast to bf16 out tiles
        nc.scalar.wait_ge(sem_small, 16)
        for ti, (s, t, co) in enumerate(tiles):
            nc.scalar.wait_ge(sem_mm, ti + 1)
            if ti >= NOUT:
                nc.scalar.wait_ge(sem_outdma, 16 * (ti - NOUT + 1))
            nc.scalar.activation(
                ot[ti % NOUT][:], psum[ti % NPS][:], Ident,
                bias=bmix_sb[:, co * SPC + s: co * SPC + s + 1],
            ).then_inc(sem_act, 1)

        # ---------------- SYNC: output DMAs
        for ti, (s, t, co) in enumerate(tiles):
            nc.sync.wait_ge(sem_act, ti + 1)
            src = ot[ti % NOUT][:].rearrange("p (r u) -> p r u", u=S)[:, :, 0:W]
            nc.sync.dma_start(
                y[s, co * 128:(co + 1) * 128,
                  t * ROWS_PER_T:(t + 1) * ROWS_PER_T, :], src,
            ).then_inc(sem_outdma, 16)
        nc.sync.wait_ge(sem_outdma, 16 * len(tiles))
    return nc


_STATE = None


def _get_state():
    global _STATE
    if _STATE is not None:
        return _STATE
    import jax
    import ml_dtypes
    from jax.sharding import Mesh, PartitionSpec as P, NamedSharding
    from jax.experimental.shard_map import shard_map
    bass, tile, mybir = _imports()
    from concourse.bass2jax import (
        install_neuronx_cc_hook, _bass_exec_p, partition_id_tensor)

    install_neuronx_cc_hook()
    nc = build_bass_raw()

    partition_name = (nc.partition_id_tensor.name
                      if nc.partition_id_tensor else None)
    in_names, out_names, out_avals = [], [], []
    for alloc in nc.m.functions[0].allocations:
        if not isinstance(alloc, mybir.MemoryLocationSet):
            continue
        name = alloc.memorylocations[0].name
        if alloc.kind == "ExternalInput":
            if name != partition_name:
                in_names.append(name)
        elif alloc.kind == "ExternalOutput":
            out_names.append(name)
            out_avals.append(jax.core.ShapedArray(
                tuple(alloc.tensor_shape), mybir.dt.np(alloc.dtype)))
    n_params = len(in_names)
    in_names_all = in_names + out_names + (
        [partition_name] if partition_name else [])

    def _body(*args):
        operands = list(args)
        if partition_name is not None:
            operands.append(partition_id_tensor())
        outs = _bass_exec_p.bind(
            *operands, out_avals=tuple(out_avals),
            in_names=tuple(in_names_all), out_names=tuple(out_names),
            lowering_input_output_aliases=(),
            sim_require_finite=True, sim_require_nnan=True, nc=nc)
        return tuple(outs)

    devices = jax.devices()[:NCORES]
    mesh = Mesh(np.asarray(devices), ("core",))
    shard = NamedSharding(mesh, P("core"))
    n_ops = n_params + len(out_names)
    sharded = jax.jit(
        shard_map(_body, mesh=mesh, in_specs=(P("core"),) * n_ops,
                  out_specs=(P("core"),) * len(out_names), check_rep=False),
        keep_unused=True)

    # Persistent device-resident dummy for the y operand: the NEFF binds
    # outputs to fresh result buffers (the kernel writes every element), so
    # the operand's contents are never read. Created on device: no upload.
    ydummy = jax.jit(
        lambda: jax.numpy.zeros((B, DIM, H, W), jax.numpy.bfloat16),
        out_shardings=shard)()
    ydummy.block_until_ready()

    _STATE = dict(jax=jax, ml_dtypes=ml_dtypes, nc=nc, sharded=sharded,
                  shard=shard, ydummy=ydummy, in_names=in_names)
    return _STATE


def kernel(x, attention, weight, bias):
    st = _get_state()
    jax, ml_dtypes = st["jax"], st["ml_dtypes"]
    bf16 = ml_dtypes.bfloat16

    # x -> bf16, start its upload first (it's the biggest input) so the
    # weight prep below overlaps with the transfer.
    x = np.asarray(x)
    xd = jax.device_put(x.astype(bf16), st["shard"])

    # Host-side weight mix: bank -> [n, ci, ky, kx, co] matmul layout once
    # (18.9 MB transpose), then one sgemm mixes all 16 samples' kernels.
    weight = np.asarray(weight, dtype=np.float32)
    attention = np.asarray(attention, dtype=np.float32)
    bankT = np.ascontiguousarray(weight.transpose(0, 2, 3, 4, 1)).reshape(
        NK, -1)
    wmix = (attention @ bankT).astype(bf16).reshape(
        B, CI_CH, 128, KK * DIM)
    wd = jax.device_put(wmix, st["shard"])

    # Host-mixed bias, laid out [core*128+p, co*SPC+s].
    bm = (attention @ np.asarray(bias, dtype=np.float32))
    bmixT = np.ascontiguousarray(
        bm.reshape(NCORES, SPC, CO_CH, 128).transpose(0, 3, 2, 1)).reshape(
        NCORES * 128, CO_CH * SPC)
    bd = jax.device_put(bmixT, st["shard"])

    (yarr,) = st["sharded"](xd, wd, bd, st["ydummy"])
    return np.asarray(yarr).astype(np.float32)


# revision 3
# speedup vs baseline: 4.5983x; 3.6910x over previous
"""Trainium2 Bass kernel: per-sample dynamic conv (KernelAggregation).

Problem: out[b] = conv2d(x[b], sum_n att[b,n]*W[n], pad=1) + (att @ bias)[b]
  x: (16, 256, 56, 56) f32, att: (16, 8), W: (8, 256, 256, 3, 3), bias: (8, 256)

Sharding: data-parallel over batch, 2 samples per core across 8 cores.

The axon tunnel moves ~40-80 MB/s and the host has a single CPU, so wall
time is wire bytes + host byte-shuffling; device compute (~0.3 ms/core) is
free. Design:
  * x ships as bf16 (25.7 MB, not 51.4); y returns as bf16.
  * The weight bank ships ONCE, sharded (9.4 MB bf16, one kernel per
    core) in its native layout; a small stage-1 jax jit all-gathers and
    transposes it on device into matmul layout. No host transpose, no
    host mixing sgemm, no 151 MB replication.
  * The Bass kernel mixes per-sample conv weights on DVE via
    scalar_tensor_tensor FMA (acc = att[s,n]*bank[n] + acc), then runs
    the conv as 9 shifted bf16 matmuls accumulated in PSUM.
  * Everything dispatches async; the only blocking point is the final y
    fetch. The shard_map jit is built once at module scope; warm calls
    pay only transfers + one dispatch chain.
  * The donated-zero output upload of run_bass_kernel_spmd's axon path is
    replaced by a persistent on-device dummy operand (the kernel writes
    every y element, so zero-init is unnecessary).
"""

import numpy as np
from contextlib import ExitStack

B, DIM, H, W = 16, 256, 56, 56
NK, KS = 8, 3
NCORES = 8
SPC = B // NCORES          # samples per core
S = W + 2                  # padded row stride (58)
NPAD = S * S               # 3364
XP_LEN = NPAD + 4          # slack so shifted reads stay in-bounds (even)
ROWS_PER_T = 8
NT = H // ROWS_PER_T       # 7 spatial tiles
NTILE = ROWS_PER_T * S     # 464 (matmul moving dim)
CI_CH = DIM // 128         # 2
CO_CH = DIM // 128         # 2
KK = KS * KS               # 9

NPS = 4     # PSUM tiles
NOUT = 4    # output staging buffers


def _imports():
    try:
        import concourse.bass as bass  # noqa: F401
    except ImportError:
        import sys
        for p in ("/opt/trn_rl_repo",):
            if p not in sys.path:
                sys.path.insert(0, p)
    import concourse.bass as bass
    import concourse.tile as tile
    from concourse import mybir
    return bass, tile, mybir


def build_bass_raw():
    bass, tile, mybir = _imports()
    dt = mybir.dt
    nc = bass.Bass()

    x = nc.dram_tensor("x", [SPC, DIM, H, W], dt.bfloat16, kind="ExternalInput")
    # Device-gathered+transposed bank from the stage-1 jit: [n, ci, kk*co].
    gbank = nc.dram_tensor("gbank", [NK, CI_CH, 128, KK * DIM], dt.bfloat16,
                           kind="ExternalInput")
    attb = nc.dram_tensor("attb", [128, SPC * NK], dt.float32,
                          kind="ExternalInput")
    bmixT = nc.dram_tensor("bmixT", [128, CO_CH * SPC], dt.float32,
                           kind="ExternalInput")
    y = nc.dram_tensor("y", [SPC, DIM, H, W], dt.bfloat16,
                       kind="ExternalOutput")

    ctx = ExitStack()
    with ctx:
        sbm = lambda shape, name: ctx.enter_context(
            nc.sbuf_tensor(name, shape, dt.bfloat16))
        sbf = lambda shape, name: ctx.enter_context(
            nc.sbuf_tensor(name, shape, dt.float32))
        att_sb = sbf([128, SPC * NK], "att_sb")
        bmix_sb = sbf([128, CO_CH * SPC], "bmix_sb")
        xp = [[sbm([128, XP_LEN], f"xp{s}_{c}") for c in range(CI_CH)]
              for s in range(SPC)]
        bank = [[sbm([128, KK * DIM], f"bk{n}_{c}") for c in range(CI_CH)]
                for n in range(NK)]
        acc = [[sbf([128, KK * DIM], f"acc{s}_{c}") for c in range(CI_CH)]
               for s in range(SPC)]
        wm = [[sbm([128, KK * DIM], f"wm{s}_{c}") for c in range(CI_CH)]
              for s in range(SPC)]
        ot = [sbm([128, NTILE], f"ot{i}") for i in range(NOUT)]
        psum = [ctx.enter_context(nc.psum_tensor(f"ps{i}", [128, NTILE],
                                                 dt.float32))
                for i in range(NPS)]

        sem = lambda name: ctx.enter_context(nc.semaphore(name))
        sem_small = sem("sem_small")   # att+bmix DMA done (2x16)
        sem_ms = sem("sem_ms")         # DVE memsets done (1 each, 4)
        sem_x = sem("sem_x")           # x interior DMA done (4x16)
        sem_bank = sem("sem_bank")     # bank DMA (n,c) done at 16*(2n+c+1)
        sem_wm = sem("sem_wm")         # mixed weights (s,c) ready (4)
        sem_mm = sem("sem_mm")         # PE per-out-tile group done (28)
        sem_act = sem("sem_act")       # ACT out bias-copies (28)
        sem_outdma = sem("sem_outdma")  # out DMA done (16 each, 28)

        Ident = mybir.ActivationFunctionType.Identity
        Alu = mybir.AluOpType

        # ---------------- GPSIMD: all input DMAs
        nc.gpsimd.dma_start(att_sb[:], attb[:, :]).then_inc(sem_small, 16)
        nc.gpsimd.dma_start(bmix_sb[:], bmixT[:, :]).then_inc(sem_small, 16)
        for n in range(NK):
            for c in range(CI_CH):
                nc.gpsimd.dma_start(bank[n][c][:],
                                    gbank[n, c, :, :]).then_inc(sem_bank, 16)
        for i, (s, c) in enumerate([(s, c) for s in range(SPC)
                                    for c in range(CI_CH)]):
            nc.gpsimd.wait_ge(sem_ms, i + 1)
            interior = xp[s][c][:, :NPAD].rearrange(
                "p (r u) -> p r u", u=S)[:, 1:1 + H, 1:1 + W]
            nc.gpsimd.dma_start(
                interior, x[s, c * 128:(c + 1) * 128, :, :]).then_inc(sem_x, 16)

        # ---------------- DVE: zero padded x images, then weight mixing
        for s in range(SPC):
            for c in range(CI_CH):
                nc.vector.memset(xp[s][c][:].bitcast(dt.float32),
                                 0.0).then_inc(sem_ms, 1)
        nc.vector.wait_ge(sem_small, 16)   # att_sb loaded
        for n in range(NK):
            for c in range(CI_CH):
                nc.vector.wait_ge(sem_bank, 16 * (2 * n + c + 1))
                for s in range(SPC):
                    a = att_sb[:, s * NK + n: s * NK + n + 1]
                    if n == 0:
                        nc.vector.tensor_scalar_mul(
                            acc[s][c][:], bank[n][c][:], a)
                    else:
                        nc.vector.scalar_tensor_tensor(
                            acc[s][c][:], bank[n][c][:], a, acc[s][c][:],
                            Alu.mult, Alu.add)
        for s in range(SPC):
            for c in range(CI_CH):
                nc.vector.tensor_copy(wm[s][c][:],
                                      acc[s][c][:]).then_inc(sem_wm, 1)

        # ---------------- PE: conv matmuls
        tiles = [(s, t, co) for s in range(SPC) for t in range(NT)
                 for co in range(CO_CH)]
        nc.tensor.wait_ge(sem_x, 16 * SPC * CI_CH)
        nc.tensor.wait_ge(sem_wm, SPC * CI_CH)
        for ti, (s, t, co) in enumerate(tiles):
            if ti >= NPS:
                nc.tensor.wait_ge(sem_act, ti - NPS + 1)
            for c in range(CI_CH):
                for kp in range(KK):
                    off = (kp // 3) * S + (kp % 3) + t * NTILE
                    lhsT = wm[s][c][:, kp * DIM + co * 128:
                                    kp * DIM + co * 128 + 128]
                    rhs = xp[s][c][:, off: off + NTILE]
                    mm = nc.tensor.matmul(
                        psum[ti % NPS][:], lhsT, rhs,
                        start=(c == 0 and kp == 0),
                        stop=(c == CI_CH - 1 and kp == KK - 1))
            mm.then_inc(sem_mm, 1)

        # ---------------- ACT: bias-add + cast to bf16 out tiles
        nc.scalar.wait_ge(sem_small, 32)
        for ti, (s, t, co) in enumerate(tiles):
            nc.scalar.wait_ge(sem_mm, ti + 1)
            if ti >= NOUT:
                nc.scalar.wait_ge(sem_outdma, 16 * (ti - NOUT + 1))
            nc.scalar.activation(
                ot[ti % NOUT][:], psum[ti % NPS][:], Ident,
                bias=bmix_sb[:, co * SPC + s: co * SPC + s + 1],
            ).then_inc(sem_act, 1)

        # ---------------- SYNC: output DMAs
        for ti, (s, t, co) in enumerate(tiles):
            nc.sync.wait_ge(sem_act, ti + 1)
            src = ot[ti % NOUT][:].rearrange("p (r u) -> p r u", u=S)[:, :, 0:W]
            nc.sync.dma_start(
                y[s, co * 128:(co + 1) * 128,
                  t * ROWS_PER_T:(t + 1) * ROWS_PER_T, :], src,
            ).then_inc(sem_outdma, 16)
        nc.sync.wait_ge(sem_outdma, 16 * len(tiles))
    return nc


_STATE = None


def _get_state():
    global _STATE
    if _STATE is not None:
        return _STATE
    import jax
    import jax.numpy as jnp
    import ml_dtypes
    from jax.sharding import Mesh, PartitionSpec as P, NamedSharding
    from jax.experimental.shard_map import shard_map
    bass, tile, mybir = _imports()
    from concourse.bass2jax import (
        install_neuronx_cc_hook, _bass_exec_p, partition_id_tensor)

    install_neuronx_cc_hook()
    nc = build_bass_raw()

    partition_name = (nc.partition_id_tensor.name
                      if nc.partition_id_tensor else None)
    in_names, out_names, out_avals = [], [], []
    for alloc in nc.m.functions[0].allocations:
        if not isinstance(alloc, mybir.MemoryLocationSet):
            continue
        name = alloc.memorylocations[0].name
        if alloc.kind == "ExternalInput":
            if name != partition_name:
                in_names.append(name)
        elif alloc.kind == "ExternalOutput":
            out_names.append(name)
            out_avals.append(jax.core.ShapedArray(
                tuple(alloc.tensor_shape), mybir.dt.np(alloc.dtype)))
    n_params = len(in_names)
    in_names_all = in_names + out_names + (
        [partition_name] if partition_name else [])

    def _body(*args):
        operands = list(args)
        if partition_name is not None:
            operands.append(partition_id_tensor())
        outs = _bass_exec_p.bind(
            *operands, out_avals=tuple(out_avals),
            in_names=tuple(in_names_all), out_names=tuple(out_names),
            lowering_input_output_aliases=(),
            sim_require_finite=True, sim_require_nnan=True, nc=nc)
        return tuple(outs)

    devices = jax.devices()[:NCORES]
    mesh = Mesh(np.asarray(devices), ("core",))
    shard = NamedSharding(mesh, P("core"))
    n_ops = n_params + len(out_names)
    sharded = jax.jit(
        shard_map(_body, mesh=mesh, in_specs=(P("core"),) * n_ops,
                  out_specs=(P("core"),) * len(out_names), check_rep=False),
        keep_unused=True)

    # Stage-1: all-gather the sharded native bank and transpose it into
    # matmul layout [n, ci, (kk co)] on device.
    def _st1(b):  # per-core (1, DIM*DIM*KK) bf16
        g = jax.lax.all_gather(b, "core", axis=0, tiled=True)  # (NK, ...)
        t = g.reshape(NK, DIM, DIM, KK).transpose(0, 2, 3, 1)  # n,ci,kk,co
        return t.reshape(NK, CI_CH, 128, KK * DIM)
    st1 = jax.jit(shard_map(_st1, mesh=mesh, in_specs=(P("core"),),
                            out_specs=P("core"), check_rep=False))

    # Persistent device-resident dummy for the y operand: the NEFF binds
    # outputs to fresh result buffers (the kernel writes every element), so
    # the operand's contents are never read. Created on device: no upload.
    ydummy = jax.jit(
        lambda: jnp.zeros((B, DIM, H, W), jnp.bfloat16),
        out_shardings=shard)()
    ydummy.block_until_ready()

    _STATE = dict(jax=jax, ml_dtypes=ml_dtypes, nc=nc, sharded=sharded,
                  st1=st1, shard=shard, ydummy=ydummy)

    # Fully warm the pipeline (compiles both jits, primes transfer paths)
    # so the caller's steady-state calls see no lazy one-time costs.
    rng = np.random.default_rng(0)
    kernel(rng.standard_normal((B, DIM, H, W), dtype=np.float32),
           rng.random((B, NK), dtype=np.float32),
           rng.standard_normal((NK, DIM, DIM, KS, KS), dtype=np.float32),
           rng.standard_normal((NK, DIM), dtype=np.float32))
    return _STATE


def kernel(x, attention, weight, bias):
    st = _get_state()
    jax, ml_dtypes = st["jax"], st["ml_dtypes"]
    bf16 = ml_dtypes.bfloat16

    # x -> bf16 and start its upload first (biggest input); the rest of
    # the host prep overlaps with the transfer.
    x = np.asarray(x)
    xd = jax.device_put(x.astype(bf16), st["shard"])

    # Native-layout bank, bf16, sharded one kernel per core (9.4 MB total
    # on the wire); gathered + transposed on device by st1.
    weight = np.asarray(weight, dtype=np.float32)
    wbd = jax.device_put(weight.reshape(NK, -1).astype(bf16), st["shard"])
    gT = st["st1"](wbd)

    attention = np.asarray(attention, dtype=np.float32)
    attb = np.ascontiguousarray(np.broadcast_to(
        attention.reshape(NCORES, 1, SPC * NK),
        (NCORES, 128, SPC * NK))).reshape(NCORES * 128, SPC * NK)
    ad = jax.device_put(attb, st["shard"])

    bm = attention @ np.asarray(bias, dtype=np.float32)
    bmixT = np.ascontiguousarray(
        bm.reshape(NCORES, SPC, CO_CH, 128).transpose(0, 3, 2, 1)).reshape(
        NCORES * 128, CO_CH * SPC)
    bd = jax.device_put(bmixT, st["shard"])

    (yarr,) = st["sharded"](xd, gT, ad, bd, st["ydummy"])
    return np.asarray(yarr).astype(np.float32)
